# revision 5
# baseline (speedup 1.0000x reference)
"""Multi-head attention (B=2, S=2048, D=1024, H=16, causal + key-padding mask)
for 8 Trainium2 NeuronCores.

Sharding: data + head parallel. Core c handles batch b = c//4 and the 4 heads
h in [4*(c%4), 4*(c%4)+4). Q/K/V/O projection weights are column/row-sliced
per core (Megatron style); the output projection partial sums are reduced on
the host (4 cores per batch), which also applies the output bias.

On-device compute layout (per core):
  qT, kT: [128, 2048] float32r per head-pair (partition = 2x64 head dims)
  v:      [128, 256]  float32r per s-tile (partition = keys, free = 4 heads x 64)
  scoresT blocks: [128 k, 512 q] via K=64 matmuls; causal at block granularity;
  diagonal blocks masked by accumulating (-1e30 * I) @ mask01[r].
  softmax: exp on ACT (bias = key-padding mask), column sums via ones-matmul,
  reciprocal + K=1 broadcast matmul, tensor_tensor normalize.
  attn@V accumulated per (head, q-group) into [64, 512] PSUM; output projection
  produces out_partial^T [1024, 2048] (host reduces across cores, adds bo).

All matmuls run in float32r (full PE rate at free-dim >= 256, ~1e-4 rel err).
"""

import os

import numpy as np

import concourse.tile as tile
import concourse.mybir as mybir
from concourse import bacc
from concourse.bass_utils import run_bass_kernel_spmd

F32 = mybir.dt.float32
F32R = mybir.dt.float32r
AF = mybir.ActivationFunctionType

B, S, D, H = 2, 2048, 1024, 16
HD = D // H            # 64 head dim
NCORES = 8
HPC = H // (NCORES // B)   # 4 heads per core
NJ = S // 128          # 16 k-tiles of 128
NG = S // 512          # 4 q-groups of 512
NDT = D // 128         # 8 d-tiles of the model dim
NEG = -1.0e30

_CACHED = {}


def _build():
    nc = bacc.Bacc("TRN2", target_bir_lowering=False, debug=False,
                   num_devices=NCORES)

    # inputs (all host-pre-arranged so every DMA is contiguous)
    qt = nc.dram_tensor("qt", [128, NDT, S], F32R, kind="ExternalInput").ap()
    kt = nc.dram_tensor("kt", [128, NDT, S], F32R, kind="ExternalInput").ap()
    vt = nc.dram_tensor("vt", [128, NDT, S], F32R, kind="ExternalInput").ap()
    wqt = nc.dram_tensor("wqt", [128, NDT, 256], F32R, kind="ExternalInput").ap()
    wkt = nc.dram_tensor("wkt", [128, NDT, 256], F32R, kind="ExternalInput").ap()
    wvt = nc.dram_tensor("wvt", [128, NDT, 256], F32R, kind="ExternalInput").ap()
    wot = nc.dram_tensor("wot", [128, 2, D], F32R, kind="ExternalInput").ap()
    bq = nc.dram_tensor("bq", [128, 2], F32, kind="ExternalInput").ap()
    bk = nc.dram_tensor("bk", [128, 2], F32, kind="ExternalInput").ap()
    bvb = nc.dram_tensor("bvb", [128, 256], F32, kind="ExternalInput").ap()
    padb = nc.dram_tensor("padb", [128, NJ], F32, kind="ExternalInput").ap()
    masks = nc.dram_tensor("masks", [128, 4, 512], F32R, kind="ExternalInput").ap()
    negi = nc.dram_tensor("negi", [128, 128], F32R, kind="ExternalInput").ap()

    attnt = nc.dram_tensor("attnt", [HPC, S, S], F32, kind="ExternalOutput").ap()
    outpt = nc.dram_tensor("outpt", [D, S], F32, kind="ExternalOutput").ap()

    with tile.TileContext(nc) as tc:
        with (
            tc.tile_pool(name="consts", bufs=1) as consts,
            tc.tile_pool(name="persist", bufs=1) as persist,
            tc.tile_pool(name="ps", bufs=4, space="PSUM") as ps,
            tc.tile_pool(name="po", bufs=2, space="PSUM") as po,
            tc.tile_pool(name="psm", bufs=2, space="PSUM") as psm,
        ):
            # ---- constants ----
            ones_col = consts.tile([128, 1], F32)
            nc.vector.memset(ones_col[:], 1.0)
            ones_col_r = consts.tile([128, 1], F32R)
            nc.vector.tensor_copy(ones_col_r[:], ones_col[:])
            ones_row = consts.tile([1, 128], F32)
            nc.vector.memset(ones_row[:], 1.0)
            ones_row_r = consts.tile([1, 128], F32R)
            nc.vector.tensor_copy(ones_row_r[:], ones_row[:])
            masks_sb = consts.tile([128, 4, 512], F32R)
            nc.sync.dma_start(masks_sb[:], masks[:])
            negi_sb = consts.tile([128, 128], F32R)
            nc.sync.dma_start(negi_sb[:], negi[:])
            padb_sb = consts.tile([128, NJ], F32)
            nc.sync.dma_start(padb_sb[:], padb[:])
            bq_sb = consts.tile([128, 2], F32)
            nc.sync.dma_start(bq_sb[:], bq[:])
            bk_sb = consts.tile([128, 2], F32)
            nc.sync.dma_start(bk_sb[:], bk[:])
            bvb_sb = consts.tile([128, 256], F32)
            nc.sync.dma_start(bvb_sb[:], bvb[:])
            wo_sb = consts.tile([128, 2, D], F32R)
            nc.sync.dma_start(wo_sb[:], wot[:])

            # ---- persistent activations ----
            qT_sb = [persist.tile([128, S], F32R, name=f"qT{p}") for p in range(2)]
            kT_sb = [persist.tile([128, S], F32R, name=f"kT{p}") for p in range(2)]
            v_sb = [persist.tile([128, 256], F32R, name=f"v{j}") for j in range(NJ)]
            oT_sb = [persist.tile([128, S], F32R, name=f"oT{p}") for p in range(2)]

            # ---- projections ----
            with (
                tc.tile_pool(name="inp", bufs=9) as inp,
                tc.tile_pool(name="wts", bufs=1) as wts,
            ):
                # V projection: v[s, d'] = sum_d VT[d, s] * wvT[d, d'] + bv
                wv_sb = wts.tile([128, NDT, 256], F32R)
                nc.sync.dma_start(wv_sb[:], wvt[:])
                vtiles = []
                for dt in range(NDT):
                    t = inp.tile([128, S], F32R, tag="inp", name=f"vt{dt}")
                    nc.sync.dma_start(t[:], vt[:, dt, :])
                    vtiles.append(t)
                for st in range(NJ):
                    pv = ps.tile([128, 512], F32, tag="mm", name="pv")
                    for dt in range(NDT):
                        nc.tensor.matmul(
                            pv[:, :256],
                            vtiles[dt][:, st * 128:(st + 1) * 128],
                            wv_sb[:, dt],
                            start=(dt == 0), stop=(dt == NDT - 1),
                        )
                    nc.vector.tensor_tensor(
                        v_sb[st][:], pv[:, :256], bvb_sb[:], mybir.AluOpType.add
                    )

                # Q/K projections: xT[d', s] = sum_d wxT[d, d'] * XT[d, s] + bx
                wq_sb = wts.tile([128, NDT, 256], F32R)
                nc.sync.dma_start(wq_sb[:], wqt[:])
                wk_sb = wts.tile([128, NDT, 256], F32R)
                nc.sync.dma_start(wk_sb[:], wkt[:])
                for which, wsb, xdram, bsb, dst in (
                    ("q", wq_sb, qt, bq_sb, qT_sb),
                    ("k", wk_sb, kt, bk_sb, kT_sb),
                ):
                    xtiles = []
                    for dt in range(NDT):
                        t = inp.tile([128, S], F32R, tag="inp",
                                     name=f"{which}t{dt}")
                        nc.sync.dma_start(t[:], xdram[:, dt, :])
                        xtiles.append(t)
                    for pair in range(2):
                        for g in range(NG):
                            px = ps.tile([128, 512], F32, tag="mm", name="px")
                            for dt in range(NDT):
                                nc.tensor.matmul(
                                    px[:],
                                    wsb[:, dt, pair * 128:(pair + 1) * 128],
                                    xtiles[dt][:, g * 512:(g + 1) * 512],
                                    start=(dt == 0), stop=(dt == NDT - 1),
                                )
                            nc.scalar.activation(
                                dst[pair][:, g * 512:(g + 1) * 512], px[:],
                                AF.Identity, bias=bsb[:, pair:pair + 1],
                            )

            # ---- attention main loop ----
            with (
                tc.tile_pool(name="exps", bufs=NJ + 1) as expp,
                tc.tile_pool(name="small", bufs=3) as small,
                tc.tile_pool(name="astage", bufs=4) as astage,
            ):
                for h in range(HPC):
                    pair, off = h // 2, 64 * (h % 2)
                    for g in range(NG):
                        J = 4 * g + 4
                        qT_h = qT_sb[pair][off:off + 64, g * 512:(g + 1) * 512]
                        psum_o = po.tile([64, 512], F32, tag="o", name="psum_o")
                        psum_sum = psm.tile([1, 512], F32, tag="sum", name="psum_sum")
                        exps = []
                        for j in range(J):
                            r = j - 4 * g
                            pss = ps.tile([128, 512], F32, tag="mm", name="pss")
                            nc.tensor.matmul(
                                pss[:],
                                kT_sb[pair][off:off + 64, j * 128:(j + 1) * 128],
                                qT_h,
                                start=True, stop=(r < 0),
                            )
                            if r >= 0:
                                nc.tensor.matmul(
                                    pss[:], negi_sb[:], masks_sb[:, r],
                                    start=False, stop=True,
                                )
                            e = expp.tile([128, 512], F32R, tag="exp", name="e")
                            nc.scalar.activation(
                                e[:], pss[:], AF.Exp,
                                bias=padb_sb[:, j:j + 1],
                            )
                            exps.append(e)
                            nc.tensor.matmul(
                                psum_o[:],
                                v_sb[j][:, 64 * h:64 * h + 64],
                                e[:],
                                start=(j == 0), stop=(j == J - 1),
                            )
                            nc.tensor.matmul(
                                psum_sum[:], ones_col_r[:], e[:],
                                start=(j == 0), stop=(j == J - 1),
                            )
                        r_sb = small.tile([1, 512], F32R, tag="r", name="r_sb")
                        with nc.allow_low_precision(reason="f32r is 4-byte"):
                            nc.vector.reciprocal(r_sb[:], psum_sum[:])
                        prb = ps.tile([128, 512], F32, tag="mm", name="prb")
                        nc.tensor.matmul(
                            prb[:], ones_row_r[:], r_sb[:], start=True, stop=True
                        )
                        rb_sb = small.tile([128, 512], F32, tag="rb", name="rb_sb")
                        nc.scalar.copy(rb_sb[:], prb[:])

                        for j, e in enumerate(exps):
                            r = j - 4 * g
                            w0 = 128 * r if r > 0 else 0
                            a = astage.tile([128, 512], F32, tag="a", name="a")
                            nc.vector.tensor_tensor(
                                a[:, w0:], e.bitcast(F32)[:, w0:], rb_sb[:, w0:],
                                mybir.AluOpType.mult,
                            )
                            nc.sync.dma_start(
                                attnt[h, j * 128:(j + 1) * 128,
                                      g * 512 + w0:(g + 1) * 512],
                                a[:, w0:],
                            )
                        nc.vector.tensor_tensor(
                            oT_sb[pair][off:off + 64, g * 512:(g + 1) * 512],
                            psum_o[:], rb_sb[:64, :], mybir.AluOpType.mult,
                        )

                # ---- output projection ----
                for mt in range(NDT):
                    for sg in range(NG):
                        pp = ps.tile([128, 512], F32, tag="mm", name="pp")
                        for dt in range(2):
                            nc.tensor.matmul(
                                pp[:],
                                wo_sb[:, dt, mt * 128:(mt + 1) * 128],
                                oT_sb[dt][:, sg * 512:(sg + 1) * 512],
                                start=(dt == 0), stop=(dt == 1),
                            )
                        o = astage.tile([128, 512], F32, tag="a", name="o")
                        nc.vector.tensor_copy(o[:], pp[:])
                        nc.sync.dma_start(
                            outpt[mt * 128:(mt + 1) * 128,
                                  sg * 512:(sg + 1) * 512],
                            o[:],
                        )

    nc.compile()
    return nc


def _rearr_dxs(x):
    # [Dm, S] -> [128, Dm//128, S] contiguous (partition-major d-tiles)
    return np.ascontiguousarray(
        x.reshape(x.shape[0] // 128, 128, x.shape[1]).transpose(1, 0, 2)
    )


def kernel(Q, K, V, attention_mask, wq, bq, wk, bk, wv, bv, wo, bo):
    Q = np.asarray(Q, np.float32)
    K = np.asarray(K, np.float32)
    V = np.asarray(V, np.float32)
    attention_mask = np.asarray(attention_mask)
    wq, bq_, wk, bk_ = (np.asarray(a, np.float32) for a in (wq, bq, wk, bk))
    wv, bv_, wo, bo_ = (np.asarray(a, np.float32) for a in (wv, bv, wo, bo))

    if "nc" not in _CACHED:
        _CACHED["nc"] = _build()
    nc = _CACHED["nc"]

    scale = 1.0 / np.sqrt(np.float32(HD))

    # constants shared by all cores
    mask01 = np.zeros((128, 4, 512), np.float32)
    p = np.arange(128)[:, None]
    f = np.arange(512)[None, :]
    for r in range(4):
        mask01[:, r, :] = (f < 128 * r + p).astype(np.float32)
    negi = (NEG * np.eye(128)).astype(np.float32)

    in_maps = []
    for c in range(NCORES):
        b = c // (NCORES // B)
        hg = c % (NCORES // B)
        sl = slice(hg * HPC * HD, (hg + 1) * HPC * HD)  # this core's 256 dims

        padbias = np.where(attention_mask[b] != 0, 0.0, NEG).astype(np.float32)
        in_maps.append({
            "qt": _rearr_dxs(Q[b].T),
            "kt": _rearr_dxs(K[b].T),
            "vt": _rearr_dxs(V[b].T),
            "wqt": _rearr_dxs(np.ascontiguousarray(wq.T[:, sl])),
            "wkt": _rearr_dxs(np.ascontiguousarray(wk.T[:, sl] * scale)),
            "wvt": _rearr_dxs(np.ascontiguousarray(wv.T[:, sl])),
            "wot": _rearr_dxs(np.ascontiguousarray(wo.T[sl, :])),
            "bq": np.ascontiguousarray(bq_[sl].reshape(2, 128).T),
            "bk": np.ascontiguousarray((bk_[sl] * scale).reshape(2, 128).T),
            "bvb": np.broadcast_to(bv_[sl], (128, 256)).copy(),
            "padb": np.ascontiguousarray(padbias.reshape(NJ, 128).T),
            "masks": mask01,
            "negi": negi,
        })

    trace = bool(os.environ.get("MHA_TRACE"))
    res = run_bass_kernel_spmd(
        nc, in_maps, core_ids=list(range(NCORES)), trace=trace
    )
    if trace:
        kernel.last_exec_time_ns = res.exec_time_ns
        kernel.last_trace = (
            res.instructions_and_trace[1] if res.instructions_and_trace else None
        )

    # ---- host gather ----
    out = np.zeros((B, S, D), np.float32)
    attn = np.zeros((B, H, S, S), np.float32)
    tril = np.tril(np.ones((S, S), bool))
    for c in range(NCORES):
        b = c // (NCORES // B)
        hg = c % (NCORES // B)
        rc = res.results[c]
        out[b] += rc["outpt"].T
        for hl in range(HPC):
            h = hg * HPC + hl
            attn[b, h] = np.where(tril, rc["attnt"][hl].T, 0.0)
    out += bo_[None, None, :]
    return out, attn


# revision 9
# speedup vs baseline: 1.3762x; 1.3762x over previous
"""Multi-head attention (B=2, S=2048, D=1024, H=16, causal + key-padding mask)
for 8 Trainium2 NeuronCores.

Sharding: data + head parallel. Core c handles batch b = c//4 and the 4 heads
h in [4*(c%4), 4*(c%4)+4). Q/K/V/O projection weights are column/row-sliced
per core (Megatron style); the output projection partial sums are reduced on
the host (4 cores per batch), which also applies the output bias.

Precision plan:
  q/k projections + scoresT matmuls: float32r (~1e-4) so the attn output
  (exp in fp32, fp32 normalize) stays at ~2e-4 scale-relative error.
  v projection, attn@V, out projection: bf16 (~3e-3 on `out` only).
Perf notes (from HW traces): f32r MM ~3 cyc/row, bf16 ~1 cyc/row. Scores are
K=64 so the two heads of a pair are interleaved j-by-j at base partitions
0/64 -> PE row-group packing runs them concurrently. Softmax column sums come
free as row 64 of the attn@V matmul (ones column appended to v). Causal mask
is applied by accumulating (-1e30*I) @ mask01[r] into the diagonal score
blocks before exp.
"""

import os

import numpy as np
import ml_dtypes

import concourse.tile as tile
import concourse.mybir as mybir
from concourse import bacc
from concourse.bass_utils import run_bass_kernel_spmd

F32 = mybir.dt.float32
F32R = mybir.dt.float32r
BF16 = mybir.dt.bfloat16
AF = mybir.ActivationFunctionType
MUL = mybir.AluOpType.mult
ADD = mybir.AluOpType.add

B, S, D, H = 2, 2048, 1024, 16
HD = D // H            # 64 head dim
NCORES = 8
HPC = H // (NCORES // B)   # 4 heads per core
NJ = S // 128          # 16 k-tiles of 128
NG = S // 512          # 4 q-groups of 512
NDT = D // 128         # 8 d-tiles of the model dim
NEG = -1.0e30

_CACHED = {}


def _build():
    nc = bacc.Bacc("TRN2", target_bir_lowering=False, debug=False,
                   num_devices=NCORES)

    qt = nc.dram_tensor("qt", [128, NDT, S], F32R, kind="ExternalInput").ap()
    kt = nc.dram_tensor("kt", [128, NDT, S], F32R, kind="ExternalInput").ap()
    vt = nc.dram_tensor("vt", [128, NDT, S], BF16, kind="ExternalInput").ap()
    wqt = nc.dram_tensor("wqt", [128, NDT, 256], F32R, kind="ExternalInput").ap()
    wkt = nc.dram_tensor("wkt", [128, NDT, 256], F32R, kind="ExternalInput").ap()
    wvt = nc.dram_tensor("wvt", [128, NDT, 256], BF16, kind="ExternalInput").ap()
    wot = nc.dram_tensor("wot", [128, 2, D], BF16, kind="ExternalInput").ap()
    bq = nc.dram_tensor("bq", [128, 2], F32, kind="ExternalInput").ap()
    bk = nc.dram_tensor("bk", [128, 2], F32, kind="ExternalInput").ap()
    bvb = nc.dram_tensor("bvb", [128, 256], F32, kind="ExternalInput").ap()
    padb = nc.dram_tensor("padb", [128, NJ], F32, kind="ExternalInput").ap()
    masks = nc.dram_tensor("masks", [128, 4, 512], BF16, kind="ExternalInput").ap()
    negi = nc.dram_tensor("negi", [128, 128], BF16, kind="ExternalInput").ap()

    attnt = nc.dram_tensor("attnt", [HPC, S, S], F32, kind="ExternalOutput").ap()
    outpt = nc.dram_tensor("outpt", [D, S], F32, kind="ExternalOutput").ap()

    with tile.TileContext(nc) as tc:
        with (
            tc.tile_pool(name="consts", bufs=1) as consts,
            tc.tile_pool(name="persist", bufs=1) as persist,
            tc.tile_pool(name="ps", bufs=5, space="PSUM") as ps,
            tc.tile_pool(name="po", bufs=3, space="PSUM") as po,
        ):
            # ---- constants ----
            ones_row = consts.tile([1, 128], F32)
            nc.vector.memset(ones_row[:], 1.0)
            ones_row_r = consts.tile([1, 128], F32R)
            nc.vector.tensor_copy(ones_row_r[:], ones_row[:])
            ones_4 = consts.tile([128, HPC, 1], F32)
            nc.vector.memset(ones_4[:], 1.0)
            masks_sb = consts.tile([128, 4, 512], BF16)
            nc.sync.dma_start(masks_sb[:], masks[:])
            negi_sb = consts.tile([128, 128], BF16)
            nc.sync.dma_start(negi_sb[:], negi[:])
            padb_sb = consts.tile([128, NJ], F32)
            nc.sync.dma_start(padb_sb[:], padb[:])
            bq_sb = consts.tile([128, 2], F32)
            nc.sync.dma_start(bq_sb[:], bq[:])
            bk_sb = consts.tile([128, 2], F32)
            nc.sync.dma_start(bk_sb[:], bk[:])
            bvb_sb = consts.tile([128, 256], F32)
            nc.sync.dma_start(bvb_sb[:], bvb[:])
            wo_sb = consts.tile([128, 2, D], BF16)
            nc.sync.dma_start(wo_sb[:], wot[:])

            # ---- persistent activations ----
            qT_sb = [persist.tile([128, S], F32R, name=f"qT{p}") for p in range(2)]
            kT_sb = [persist.tile([128, S], F32R, name=f"kT{p}") for p in range(2)]
            # v with an appended ones column per head: [k, head, 65]
            v_sb = [persist.tile([128, HPC, HD + 1], BF16, name=f"v{j}")
                    for j in range(NJ)]
            oT_sb = [persist.tile([128, S], BF16, name=f"oT{p}") for p in range(2)]

            # ---- projections ----
            with (
                tc.tile_pool(name="inp", bufs=9) as inp,
                tc.tile_pool(name="wts", bufs=1) as wts,
            ):
                # V projection: v[s, d'] = sum_d VT[d, s] * wvT[d, d'] + bv
                wv_sb = wts.tile([128, NDT, 256], BF16)
                nc.sync.dma_start(wv_sb[:], wvt[:])
                vtiles = []
                for dt in range(NDT):
                    t = inp.tile([128, S], BF16, tag="binp", name=f"vt{dt}")
                    nc.sync.dma_start(t[:], vt[:, dt, :])
                    vtiles.append(t)
                for st in range(NJ):
                    pv = ps.tile([128, 512], F32, tag="mm", name="pv")
                    for dt in range(NDT):
                        nc.tensor.matmul(
                            pv[:, :256],
                            vtiles[dt][:, st * 128:(st + 1) * 128],
                            wv_sb[:, dt],
                            start=(dt == 0), stop=(dt == NDT - 1),
                        )
                    nc.vector.tensor_tensor(
                        v_sb[st][:, :, :HD],
                        pv[:, :256].rearrange("p (h d) -> p h d", h=HPC),
                        bvb_sb.rearrange("p (h d) -> p h d", h=HPC),
                        ADD,
                    )
                    nc.vector.tensor_copy(v_sb[st][:, :, HD:HD + 1], ones_4[:])

                # Q/K projections: xT[d', s] = sum_d wxT[d, d'] * XT[d, s] + bx
                wq_sb = wts.tile([128, NDT, 256], F32R)
                nc.sync.dma_start(wq_sb[:], wqt[:])
                wk_sb = wts.tile([128, NDT, 256], F32R)
                nc.sync.dma_start(wk_sb[:], wkt[:])
                for which, wsb, xdram, bsb, dst in (
                    ("q", wq_sb, qt, bq_sb, qT_sb),
                    ("k", wk_sb, kt, bk_sb, kT_sb),
                ):
                    xtiles = []
                    for dt in range(NDT):
                        t = inp.tile([128, S], F32R, tag="inp",
                                     name=f"{which}t{dt}")
                        nc.sync.dma_start(t[:], xdram[:, dt, :])
                        xtiles.append(t)
                    for pair in range(2):
                        for g in range(NG):
                            px = ps.tile([128, 512], F32, tag="mm", name="px")
                            for dt in range(NDT):
                                nc.tensor.matmul(
                                    px[:],
                                    wsb[:, dt, pair * 128:(pair + 1) * 128],
                                    xtiles[dt][:, g * 512:(g + 1) * 512],
                                    start=(dt == 0), stop=(dt == NDT - 1),
                                )
                            nc.scalar.activation(
                                dst[pair][:, g * 512:(g + 1) * 512], px[:],
                                AF.Identity, bias=bsb[:, pair:pair + 1],
                            )

            # ---- attention main loop (head pairs interleaved for PE
            #      row-group packing of the K=64 score matmuls) ----
            with (
                tc.tile_pool(name="exps", bufs=2 * NJ + 1) as expp,
                tc.tile_pool(name="ebf", bufs=6) as ebfp,
                tc.tile_pool(name="small", bufs=4) as small,
                tc.tile_pool(name="astage", bufs=4) as astage,
            ):
                for pair in range(2):
                    for g in range(NG):
                        J = 4 * g + 4
                        exps = {0: [], 1: []}
                        psum_o = {}
                        for sub in range(2):
                            psum_o[sub] = po.tile(
                                [HD + 1, 512], F32, tag="o", name=f"po{sub}"
                            )
                        for j in range(J):
                            r = j - 4 * g
                            ebfs = {}
                            for sub in range(2):
                                off = 64 * sub
                                h = 2 * pair + sub
                                pss = ps.tile([128, 512], F32, tag="mm",
                                              name="pss")
                                nc.tensor.matmul(
                                    pss[:],
                                    kT_sb[pair][off:off + 64,
                                                j * 128:(j + 1) * 128],
                                    qT_sb[pair][off:off + 64,
                                                g * 512:(g + 1) * 512],
                                    start=True, stop=(r < 0),
                                )
                                if r >= 0:
                                    nc.tensor.matmul(
                                        pss[:, :128 * (r + 1)],
                                        negi_sb[:],
                                        masks_sb[:, r, :128 * (r + 1)],
                                        start=False, stop=True,
                                    )
                                e = expp.tile([128, 512], F32, tag="exp",
                                              name="e")
                                nc.scalar.activation(
                                    e[:], pss[:], AF.Exp,
                                    bias=padb_sb[:, j:j + 1],
                                )
                                exps[sub].append(e)
                                ebf = ebfp.tile([128, 512], BF16, tag="ebf",
                                                name="ebf")
                                nc.vector.tensor_copy(ebf[:], e[:])
                                ebfs[sub] = ebf
                            for sub in range(2):
                                h = 2 * pair + sub
                                nc.tensor.matmul(
                                    psum_o[sub][:],
                                    v_sb[j][:, h],
                                    ebfs[sub][:],
                                    start=(j == 0), stop=(j == J - 1),
                                )
                        for sub in range(2):
                            off = 64 * sub
                            s_f = small.tile([1, 512], F32, tag="sf", name="s_f")
                            nc.scalar.copy(s_f[:], psum_o[sub][HD:HD + 1, :])
                            r_f = small.tile([1, 512], F32, tag="rf", name="r_f")
                            nc.vector.reciprocal_approx_fast(r_f[:], s_f[:])
                            r_r = small.tile([1, 512], F32R, tag="rr", name="r_r")
                            nc.vector.tensor_copy(r_r[:], r_f[:])
                            prb = ps.tile([128, 512], F32, tag="mm", name="prb")
                            nc.tensor.matmul(
                                prb[:], ones_row_r[:], r_r[:],
                                start=True, stop=True,
                            )
                            rb_sb = small.tile([128, 512], F32, tag="rb",
                                               name="rb_sb")
                            nc.scalar.copy(rb_sb[:], prb[:])

                            h = 2 * pair + sub
                            for j, e in enumerate(exps[sub]):
                                r = j - 4 * g
                                w0 = 128 * r if r > 0 else 0
                                a = astage.tile([128, 512], F32, tag="a",
                                                name="a")
                                nc.vector.tensor_tensor(
                                    a[:, w0:], e[:, w0:], rb_sb[:, w0:], MUL
                                )
                                nc.sync.dma_start(
                                    attnt[h, j * 128:(j + 1) * 128,
                                          g * 512 + w0:(g + 1) * 512],
                                    a[:, w0:],
                                )
                            nc.vector.tensor_tensor(
                                oT_sb[pair][off:off + 64,
                                            g * 512:(g + 1) * 512],
                                psum_o[sub][:HD, :], rb_sb[:HD, :], MUL,
                            )

                # ---- output projection (bf16) ----
                for mt in range(NDT):
                    for sg in range(NG):
                        pp = ps.tile([128, 512], F32, tag="mm", name="pp")
                        for dt in range(2):
                            nc.tensor.matmul(
                                pp[:],
                                wo_sb[:, dt, mt * 128:(mt + 1) * 128],
                                oT_sb[dt][:, sg * 512:(sg + 1) * 512],
                                start=(dt == 0), stop=(dt == 1),
                            )
                        o = astage.tile([128, 512], F32, tag="a", name="o")
                        nc.scalar.copy(o[:], pp[:])
                        nc.sync.dma_start(
                            outpt[mt * 128:(mt + 1) * 128,
                                  sg * 512:(sg + 1) * 512],
                            o[:],
                        )

    nc.compile()
    return nc


def _rearr_dxs(x, dtype=np.float32):
    # [Dm, S] -> [128, Dm//128, S] contiguous (partition-major d-tiles)
    return np.ascontiguousarray(
        x.reshape(x.shape[0] // 128, 128, x.shape[1]).transpose(1, 0, 2)
    ).astype(dtype)


def kernel(Q, K, V, attention_mask, wq, bq, wk, bk, wv, bv, wo, bo):
    Q = np.asarray(Q, np.float32)
    K = np.asarray(K, np.float32)
    V = np.asarray(V, np.float32)
    attention_mask = np.asarray(attention_mask)
    wq, bq_, wk, bk_ = (np.asarray(a, np.float32) for a in (wq, bq, wk, bk))
    wv, bv_, wo, bo_ = (np.asarray(a, np.float32) for a in (wv, bv, wo, bo))

    if "nc" not in _CACHED:
        _CACHED["nc"] = _build()
    nc = _CACHED["nc"]

    scale = 1.0 / np.sqrt(np.float32(HD))
    bf16 = ml_dtypes.bfloat16

    mask01 = np.zeros((128, 4, 512), np.float32)
    p = np.arange(128)[:, None]
    f = np.arange(512)[None, :]
    for r in range(4):
        mask01[:, r, :] = (f < 128 * r + p).astype(np.float32)
    mask01 = mask01.astype(bf16)
    negi = (NEG * np.eye(128)).astype(bf16)

    in_maps = []
    for c in range(NCORES):
        b = c // (NCORES // B)
        hg = c % (NCORES // B)
        sl = slice(hg * HPC * HD, (hg + 1) * HPC * HD)  # this core's 256 dims

        padbias = np.where(attention_mask[b] != 0, 0.0, NEG).astype(np.float32)
        in_maps.append({
            "qt": _rearr_dxs(Q[b].T),
            "kt": _rearr_dxs(K[b].T),
            "vt": _rearr_dxs(V[b].T, bf16),
            "wqt": _rearr_dxs(np.ascontiguousarray(wq.T[:, sl])),
            "wkt": _rearr_dxs(np.ascontiguousarray(wk.T[:, sl] * scale)),
            "wvt": _rearr_dxs(np.ascontiguousarray(wv.T[:, sl]), bf16),
            "wot": _rearr_dxs(np.ascontiguousarray(wo.T[sl, :]), bf16),
            "bq": np.ascontiguousarray(bq_[sl].reshape(2, 128).T),
            "bk": np.ascontiguousarray((bk_[sl] * scale).reshape(2, 128).T),
            "bvb": np.broadcast_to(bv_[sl], (128, 256)).copy(),
            "padb": np.ascontiguousarray(padbias.reshape(NJ, 128).T),
            "masks": mask01,
            "negi": negi,
        })

    trace = bool(os.environ.get("MHA_TRACE"))
    res = run_bass_kernel_spmd(
        nc, in_maps, core_ids=list(range(NCORES)), trace=trace
    )
    if trace:
        kernel.last_exec_time_ns = res.exec_time_ns
        kernel.last_trace = (
            res.instructions_and_trace[1] if res.instructions_and_trace else None
        )

    # ---- host gather ----
    out = np.zeros((B, S, D), np.float32)
    attn = np.zeros((B, H, S, S), np.float32)
    tril = np.tril(np.ones((S, S), bool))
    for c in range(NCORES):
        b = c // (NCORES // B)
        hg = c % (NCORES // B)
        rc = res.results[c]
        out[b] += rc["outpt"].T
        for hl in range(HPC):
            h = hg * HPC + hl
            attn[b, h] = np.where(tril, rc["attnt"][hl].T, 0.0)
    out += bo_[None, None, :]
    return out, attn


# revision 14
# speedup vs baseline: 1.5577x; 1.1319x over previous
"""Multi-head attention (B=2, S=2048, D=1024, H=16, causal + key-padding mask)
for 8 Trainium2 NeuronCores.

Sharding: data + head parallel. Core c handles batch b = c//4 and the 4 heads
h in [4*(c%4), 4*(c%4)+4). Q/K/V/O projection weights are column/row-sliced
per core (Megatron style); the output projection partial sums are reduced on
the host (4 cores per batch), which also applies the output bias.

Data tier: fp16 (full PE rate, 10-bit mantissa -> ~1e-3 scale-relative
error); all accumulation fp32 in PSUM, softmax exp/normalization arithmetic
fp32 internally. The softmax numerator and denominator both come from the
same fp16 exp tile, so short-row quantization errors cancel.

Per-core layouts (all DMAs contiguous):
  qT, kT [128, 2048] per head-pair (partition = 2x64 head dims); v with an
  appended ones column [128k, 4h, 65]; scoresT [k, q] blocks of [128, 512]
  computed two-at-a-time into one [128, 1024] PSUM pair so each ACT exp
  covers 1024 columns; the ones column makes row 64 of the attn@V PSUM the
  softmax denominator for free. Causal masking accumulates (-30000*I) @
  mask01[r] into diagonal score blocks before exp; the key-padding mask is
  the per-partition exp bias. The two heads of a pair are interleaved at
  base partitions 0/64 so the K=64 score matmuls pack into disjoint PE
  row-groups. reciprocal_approx_fast + a K=1 ones matmul broadcasts the
  reciprocal sums across partitions for the final normalize.
"""

import os

import numpy as np

import concourse.tile as tile
import concourse.mybir as mybir
from concourse import bacc
from concourse.bass_utils import run_bass_kernel_spmd

F32 = mybir.dt.float32
F16 = mybir.dt.float16
AF = mybir.ActivationFunctionType
MUL = mybir.AluOpType.mult
ADD = mybir.AluOpType.add

B, S, D, H = 2, 2048, 1024, 16
HD = D // H            # 64 head dim
NCORES = 8
HPC = H // (NCORES // B)   # 4 heads per core
NJ = S // 128          # 16 k-tiles of 128
NG = S // 512          # 4 q-groups of 512
NDT = D // 128         # 8 d-tiles of the model dim
NEG = -30000.0         # fp16-representable; exp(s + NEG) == 0
NEGPAD = -1.0e30       # fp32 bias for padded keys

_CACHED = {}


def _build(padded=False):
    nc = bacc.Bacc("TRN2", target_bir_lowering=False, debug=False,
                   num_devices=NCORES)

    qt = nc.dram_tensor("qt", [128, NDT, S], F16, kind="ExternalInput").ap()
    kt = nc.dram_tensor("kt", [128, NDT, S], F16, kind="ExternalInput").ap()
    vt = nc.dram_tensor("vt", [128, NDT, S], F16, kind="ExternalInput").ap()
    wqt = nc.dram_tensor("wqt", [128, NDT, 256], F16, kind="ExternalInput").ap()
    wkt = nc.dram_tensor("wkt", [128, NDT, 256], F16, kind="ExternalInput").ap()
    wvt = nc.dram_tensor("wvt", [128, NDT, 256], F16, kind="ExternalInput").ap()
    wot = nc.dram_tensor("wot", [128, 2, D], F16, kind="ExternalInput").ap()
    bq = nc.dram_tensor("bq", [128, 2], F32, kind="ExternalInput").ap()
    bk = nc.dram_tensor("bk", [128, 2], F32, kind="ExternalInput").ap()
    bvb = nc.dram_tensor("bvb", [128, 256], F32, kind="ExternalInput").ap()
    padb = nc.dram_tensor("padb", [128, NJ], F32, kind="ExternalInput").ap()
    masks = nc.dram_tensor("masks", [128, 4, 512], F16, kind="ExternalInput").ap()
    negi = nc.dram_tensor("negi", [128, 128], F16, kind="ExternalInput").ap()

    attnt = nc.dram_tensor("attnt", [HPC, S, S], F16, kind="ExternalOutput").ap()
    outpt = nc.dram_tensor("outpt", [D, S], F32, kind="ExternalOutput").ap()

    with tile.TileContext(nc) as tc:
        with (
            tc.tile_pool(name="consts", bufs=1) as consts,
            tc.tile_pool(name="persist", bufs=1) as persist,
            tc.tile_pool(name="ps", bufs=3, space="PSUM") as ps,
            tc.tile_pool(name="po", bufs=2, space="PSUM") as po,
        ):
            # ---- constants ----
            ones_row = consts.tile([1, 128], F32)
            nc.vector.memset(ones_row[:], 1.0)
            ones_row_h = consts.tile([1, 128], F16)
            nc.vector.tensor_copy(ones_row_h[:], ones_row[:])
            ones_4 = consts.tile([128, HPC, 1], F32)
            nc.vector.memset(ones_4[:], 1.0)
            masks_sb = consts.tile([128, 4, 512], F16)
            nc.sync.dma_start(masks_sb[:], masks[:])
            negi_sb = consts.tile([128, 128], F16)
            nc.sync.dma_start(negi_sb[:], negi[:])
            padb_sb = consts.tile([128, NJ], F32)
            nc.sync.dma_start(padb_sb[:], padb[:])
            bq_sb = consts.tile([128, 2], F32)
            nc.sync.dma_start(bq_sb[:], bq[:])
            bk_sb = consts.tile([128, 2], F32)
            nc.sync.dma_start(bk_sb[:], bk[:])
            bvb_sb = consts.tile([128, 256], F32)
            nc.sync.dma_start(bvb_sb[:], bvb[:])
            wo_sb = consts.tile([128, 2, D], F16)
            nc.sync.dma_start(wo_sb[:], wot[:])

            # ---- persistent activations ----
            qT_sb = [persist.tile([128, S], F16, name=f"qT{p}") for p in range(2)]
            kT_sb = [persist.tile([128, S], F16, name=f"kT{p}") for p in range(2)]
            v_sb = [persist.tile([128, HPC, HD + 1], F16, name=f"v{j}")
                    for j in range(NJ)]
            oT_sb = [persist.tile([128, S], F16, name=f"oT{p}") for p in range(2)]

            # ---- projections ----
            with (
                tc.tile_pool(name="inp", bufs=9) as inp,
                tc.tile_pool(name="wts", bufs=1) as wts,
            ):
                # V projection: v[s, d'] = sum_d VT[d, s] * wvT[d, d'] + bv
                wv_sb = wts.tile([128, NDT, 256], F16)
                nc.sync.dma_start(wv_sb[:], wvt[:])
                vtiles = []
                for dt in range(NDT):
                    t = inp.tile([128, S], F16, tag="inp", name=f"vt{dt}")
                    nc.sync.dma_start(t[:], vt[:, dt, :])
                    vtiles.append(t)
                for st in range(NJ):
                    pv = ps.tile([128, 1024], F32, tag="mm", name="pv")
                    for dt in range(NDT):
                        nc.tensor.matmul(
                            pv[:, :256],
                            vtiles[dt][:, st * 128:(st + 1) * 128],
                            wv_sb[:, dt],
                            start=(dt == 0), stop=(dt == NDT - 1),
                        )
                    nc.vector.tensor_tensor(
                        v_sb[st][:, :, :HD],
                        pv[:, :256].rearrange("p (h d) -> p h d", h=HPC),
                        bvb_sb.rearrange("p (h d) -> p h d", h=HPC),
                        ADD,
                    )
                    nc.vector.tensor_copy(v_sb[st][:, :, HD:HD + 1], ones_4[:])

                # Q/K projections: xT[d', s] = sum_d wxT[d, d'] * XT[d, s] + bx
                wq_sb = wts.tile([128, NDT, 256], F16)
                nc.sync.dma_start(wq_sb[:], wqt[:])
                wk_sb = wts.tile([128, NDT, 256], F16)
                nc.sync.dma_start(wk_sb[:], wkt[:])
                for which, wsb, xdram, bsb, dst in (
                    ("q", wq_sb, qt, bq_sb, qT_sb),
                    ("k", wk_sb, kt, bk_sb, kT_sb),
                ):
                    xtiles = []
                    for dt in range(NDT):
                        t = inp.tile([128, S], F16, tag="inp",
                                     name=f"{which}t{dt}")
                        nc.sync.dma_start(t[:], xdram[:, dt, :])
                        xtiles.append(t)
                    for pair in range(2):
                        for g in range(NG):
                            px = ps.tile([128, 1024], F32, tag="mm", name="px")
                            for dt in range(NDT):
                                nc.tensor.matmul(
                                    px[:, :512],
                                    wsb[:, dt, pair * 128:(pair + 1) * 128],
                                    xtiles[dt][:, g * 512:(g + 1) * 512],
                                    start=(dt == 0), stop=(dt == NDT - 1),
                                )
                            nc.scalar.activation(
                                dst[pair][:, g * 512:(g + 1) * 512],
                                px[:, :512],
                                AF.Identity, bias=bsb[:, pair:pair + 1],
                            )

            # ---- attention main loop ----
            with (
                tc.tile_pool(name="exps", bufs=2 * (NJ // 2) + 1) as expp,
                tc.tile_pool(name="small", bufs=4) as small,
                tc.tile_pool(name="astage", bufs=6) as astage,
            ):
                for pair in range(2):
                    for g in range(NG):
                        J = 4 * g + 4
                        exps = {0: [], 1: []}
                        psum_o = {
                            sub: po.tile([HD + 1, 512], F32, tag="o",
                                         name=f"po{sub}")
                            for sub in range(2)
                        }
                        for j0 in range(0, J, 2):
                            for sub in range(2):
                                off = 64 * sub
                                pss = ps.tile([128, 1024], F32, tag="mm",
                                              name="pss")
                                for jj in range(2):
                                    j = j0 + jj
                                    r = j - 4 * g
                                    half = pss[:, jj * 512:(jj + 1) * 512]
                                    nc.tensor.matmul(
                                        half,
                                        kT_sb[pair][off:off + 64,
                                                    j * 128:(j + 1) * 128],
                                        qT_sb[pair][off:off + 64,
                                                    g * 512:(g + 1) * 512],
                                        start=True, stop=(r < 0),
                                    )
                                    if r >= 0:
                                        nc.tensor.matmul(
                                            pss[:, jj * 512:
                                                jj * 512 + 128 * (r + 1)],
                                            negi_sb[:],
                                            masks_sb[:, r, :128 * (r + 1)],
                                            start=False, stop=True,
                                        )
                                e = expp.tile([128, 1024], F16, tag="exp",
                                              name="e")
                                if padded:
                                    # per-k-tile padding bias differs between
                                    # the two halves
                                    for jj in range(2):
                                        nc.scalar.activation(
                                            e[:, jj * 512:(jj + 1) * 512],
                                            pss[:, jj * 512:(jj + 1) * 512],
                                            AF.Exp,
                                            bias=padb_sb[:, j0 + jj:j0 + jj + 1],
                                        )
                                else:
                                    nc.scalar.activation(
                                        e[:], pss[:], AF.Exp, bias=0.0,
                                    )
                                exps[sub].append(e)
                            for sub in range(2):
                                h = 2 * pair + sub
                                e = exps[sub][-1]
                                for jj in range(2):
                                    j = j0 + jj
                                    nc.tensor.matmul(
                                        psum_o[sub][:],
                                        v_sb[j][:, h],
                                        e[:, jj * 512:(jj + 1) * 512],
                                        start=(j == 0), stop=(j == J - 1),
                                    )
                        for sub in range(2):
                            off = 64 * sub
                            s_f = small.tile([1, 512], F32, tag="sf",
                                             name="s_f")
                            nc.scalar.copy(s_f[:], psum_o[sub][HD:HD + 1, :])
                            r_f = small.tile([1, 512], F32, tag="rf",
                                             name="r_f")
                            nc.vector.reciprocal_approx_fast(r_f[:], s_f[:])
                            r_h = small.tile([1, 512], F16, tag="rh",
                                             name="r_h")
                            nc.vector.tensor_copy(r_h[:], r_f[:])
                            prb = ps.tile([128, 1024], F32, tag="mm",
                                          name="prb")
                            nc.tensor.matmul(
                                prb[:, :512], ones_row_h[:], r_h[:],
                                start=True, stop=True,
                            )
                            rb_sb = small.tile([128, 512], F16, tag="rb",
                                               name="rb_sb")
                            nc.scalar.copy(rb_sb[:], prb[:, :512])

                            h = 2 * pair + sub
                            for jp, e in enumerate(exps[sub]):
                                for jj in range(2):
                                    j = 2 * jp + jj
                                    r = j - 4 * g
                                    w0 = 128 * r if r > 0 else 0
                                    a = astage.tile([128, 512], F16, tag="a",
                                                    name="a")
                                    nc.vector.tensor_tensor(
                                        a[:, w0:],
                                        e[:, jj * 512 + w0:(jj + 1) * 512],
                                        rb_sb[:, w0:], MUL,
                                    )
                                    nc.sync.dma_start(
                                        attnt[h, j * 128:(j + 1) * 128,
                                              g * 512 + w0:(g + 1) * 512],
                                        a[:, w0:],
                                    )
                            nc.vector.tensor_tensor(
                                oT_sb[pair][off:off + 64,
                                            g * 512:(g + 1) * 512],
                                psum_o[sub][:HD, :], rb_sb[:HD, :], MUL,
                            )

                # ---- output projection ----
                for mt in range(NDT):
                    for sg in range(NG):
                        pp = ps.tile([128, 1024], F32, tag="mm", name="pp")
                        for dt in range(2):
                            nc.tensor.matmul(
                                pp[:, :512],
                                wo_sb[:, dt, mt * 128:(mt + 1) * 128],
                                oT_sb[dt][:, sg * 512:(sg + 1) * 512],
                                start=(dt == 0), stop=(dt == 1),
                            )
                        o = astage.tile([128, 512], F32, tag="of", name="o")
                        nc.scalar.copy(o[:], pp[:, :512])
                        nc.sync.dma_start(
                            outpt[mt * 128:(mt + 1) * 128,
                                  sg * 512:(sg + 1) * 512],
                            o[:],
                        )

    nc.compile()
    return nc


def _rearr_dxs(x, dtype=np.float16):
    # [Dm, S] -> [128, Dm//128, S] contiguous (partition-major d-tiles)
    return np.ascontiguousarray(
        x.reshape(x.shape[0] // 128, 128, x.shape[1]).transpose(1, 0, 2)
    ).astype(dtype)


def kernel(Q, K, V, attention_mask, wq, bq, wk, bk, wv, bv, wo, bo):
    Q = np.asarray(Q, np.float32)
    K = np.asarray(K, np.float32)
    V = np.asarray(V, np.float32)
    attention_mask = np.asarray(attention_mask)
    wq, bq_, wk, bk_ = (np.asarray(a, np.float32) for a in (wq, bq, wk, bk))
    wv, bv_, wo, bo_ = (np.asarray(a, np.float32) for a in (wv, bv, wo, bo))

    padded = bool(np.any(np.asarray(attention_mask) == 0))
    key = ("nc", padded)
    if key not in _CACHED:
        _CACHED[key] = _build(padded=padded)
    nc = _CACHED[key]

    scale = 1.0 / np.sqrt(np.float32(HD))

    mask01 = np.zeros((128, 4, 512), np.float32)
    p = np.arange(128)[:, None]
    f = np.arange(512)[None, :]
    for r in range(4):
        mask01[:, r, :] = (f < 128 * r + p).astype(np.float32)
    mask01 = mask01.astype(np.float16)
    negi = (NEG * np.eye(128)).astype(np.float16)

    in_maps = []
    for c in range(NCORES):
        b = c // (NCORES // B)
        hg = c % (NCORES // B)
        sl = slice(hg * HPC * HD, (hg + 1) * HPC * HD)  # this core's 256 dims

        padbias = np.where(attention_mask[b] != 0, 0.0, NEGPAD).astype(np.float32)
        in_maps.append({
            "qt": _rearr_dxs(Q[b].T),
            "kt": _rearr_dxs(K[b].T),
            "vt": _rearr_dxs(V[b].T),
            "wqt": _rearr_dxs(np.ascontiguousarray(wq.T[:, sl])),
            "wkt": _rearr_dxs(np.ascontiguousarray(wk.T[:, sl] * scale)),
            "wvt": _rearr_dxs(np.ascontiguousarray(wv.T[:, sl])),
            "wot": _rearr_dxs(np.ascontiguousarray(wo.T[sl, :])),
            "bq": np.ascontiguousarray(bq_[sl].reshape(2, 128).T),
            "bk": np.ascontiguousarray((bk_[sl] * scale).reshape(2, 128).T),
            "bvb": np.broadcast_to(bv_[sl], (128, 256)).copy(),
            "padb": np.ascontiguousarray(padbias.reshape(NJ, 128).T),
            "masks": mask01,
            "negi": negi,
        })

    trace = bool(os.environ.get("MHA_TRACE"))
    res = run_bass_kernel_spmd(
        nc, in_maps, core_ids=list(range(NCORES)), trace=trace
    )
    if trace:
        kernel.last_exec_time_ns = res.exec_time_ns
        kernel.last_trace = (
            res.instructions_and_trace[1] if res.instructions_and_trace else None
        )

    # ---- host gather ----
    out = np.zeros((B, S, D), np.float32)
    attn = np.zeros((B, H, S, S), np.float32)
    tril = np.tril(np.ones((S, S), bool))
    for c in range(NCORES):
        b = c // (NCORES // B)
        hg = c % (NCORES // B)
        rc = res.results[c]
        out[b] += rc["outpt"].T
        for hl in range(HPC):
            h = hg * HPC + hl
            attn[b, h] = np.where(tril, rc["attnt"][hl].astype(np.float32).T, 0.0)
    out += bo_[None, None, :]
    return out, attn
